# revision 20
# baseline (speedup 1.0000x reference)
import os
import sys
import types

import numpy as np

sys.path.insert(0, "/opt/trn_rl_repo")

import ml_dtypes  # noqa: E402
import concourse.mybir as mybir  # noqa: E402
import concourse.tile as tile  # noqa: E402
from concourse import bacc  # noqa: E402
from concourse.bass import ds, ts  # noqa: E402
from concourse.bass_utils import run_bass_kernel_spmd  # noqa: E402

BF16 = mybir.dt.bfloat16
F32 = mybir.dt.float32
bfdt = ml_dtypes.bfloat16
AF = mybir.ActivationFunctionType
ALU = mybir.AluOpType

B, D, N = 4, 512, 2048
H, KVH, DH = 8, 2, 64
CONTEXT_LEN = 4096
NLOC = 1024  # query tokens per core
P = 128
NCORES = 8
HP = H // 2  # head pairs
NCH = N // P  # 16 key chunks of 128
NQ = N // 512  # 4 token quarters

_CACHE = {}


def _enable_trace_hook():
    """Register the NTFF profile hook (missing antenv.axon_hooks shim)."""
    try:
        import antenv

        if "antenv.axon_hooks" in sys.modules:
            return
        mod = types.ModuleType("antenv.axon_hooks")

        def set_axon_ntff_profile_hook(h):
            mod._hook = h

        def get_axon_ntff_profile_hook():
            return getattr(mod, "_hook", None)

        mod.set_axon_ntff_profile_hook = set_axon_ntff_profile_hook
        mod.get_axon_ntff_profile_hook = get_axon_ntff_profile_hook
        sys.modules["antenv.axon_hooks"] = mod
        antenv.axon_hooks = mod
        from trn_agent_boot.trn_boot import _ntff_profile_via_ctypes

        set_axon_ntff_profile_hook(_ntff_profile_via_ctypes("/opt/axon/libaxon_pjrt.so"))
    except Exception:
        pass


def _build(TRIVIAL_GB, TRIVIAL_BO):
    # Each core computes attention outputs for its local 1024 query tokens
    # (kernel columns 0-1023; the host permutes tokens so the local half is
    # first) against all 2048 keys. K/V for all 2048 tokens are computed
    # locally (redundantly across the core pair) -- no collectives. All
    # PSUM-using work flows through one sc pool so LN/projections interleave
    # with the attention stream as hooks, keeping the PE dense.
    nc = bacc.Bacc(None, target_bir_lowering=False, debug=False)
    dp = nc.declare_dram_parameter

    x_e = dp("x", [4, P, N], F32, isOutput=False)
    wq_e = dp("wq", [P, 4, 512], BF16, isOutput=False)
    wk_e = dp("wk", [P, 4, 128], BF16, isOutput=False)
    wv_e = dp("wv", [P, 4, 128], BF16, isOutput=False)
    wo_e = dp("wo", [P, 4, 512], BF16, isOutput=False)
    perm_e = dp("perm", [P, 128], BF16, isOutput=False)
    # onesi[:, 0, 0] = onesi[:, 1, 64] = 1/512 -- stats matmuls write the
    # two quarters' means into partitions 0 and 64 (32-aligned) of one tile
    onesi_e = dp("onesi", [P, 2, 65], BF16, isOutput=False)
    cq_e = dp("cq", [P, NLOC], BF16, isOutput=False)
    sq_e = dp("sq", [P, NLOC], BF16, isOutput=False)
    ck_e = dp("ck", [P, N], BF16, isOutput=False)
    sk_e = dp("sk", [P, N], BF16, isOutput=False)
    gam_e = dp("gam", [P, 4], F32, isOutput=False)
    bet_e = dp("bet", [P, 4], F32, isOutput=False)
    bo_e = dp("bo", [P, 4], F32, isOutput=False)
    out_e = dp("out", [4, P, NLOC], F32, isOutput=True)

    with tile.TileContext(nc) as tc:
        with (
            tc.tile_pool(name="persist", bufs=1) as PS,
            tc.tile_pool(name="xp", bufs=1) as XP,
            tc.tile_pool(name="tmp", bufs=2) as TMP,
            tc.tile_pool(name="tmp4", bufs=4) as TMP4,
            tc.tile_pool(name="exp", bufs=3) as EXPP,
            tc.tile_pool(name="ps_sc", bufs=2, space="PSUM") as PSC,
            tc.tile_pool(name="ps_av", bufs=1, space="PSUM") as PAV,
        ):
            # ---------------- input DMA over three issue queues ----------
            wk_sb = PS.tile([P, 4, 128], BF16, name="wk")
            wv_sb = PS.tile([P, 4, 128], BF16, name="wv")
            perm_sb = PS.tile([P, 128], BF16, name="perm")
            onesi_sb = PS.tile([P, 2, 65], BF16, name="onesi")
            gam_sb = PS.tile([P, 4], F32, name="gam")
            bet_sb = PS.tile([P, 4], F32, name="bet")
            bo_sb = PS.tile([P, 4], F32, name="bo")
            wq_sb = PS.tile([P, 4, 512], BF16, name="wq")
            cq_sb = PS.tile([P, NLOC], BF16, name="cq")
            sq_sb = PS.tile([P, NLOC], BF16, name="sq")
            ck_sb = PS.tile([P, N], BF16, name="ck")
            sk_sb = PS.tile([P, N], BF16, name="sk")
            wo_sb = PS.tile([P, 4, 512], BF16, name="wo")
            for t, e in (
                (wk_sb, wk_e), (wv_sb, wv_e), (perm_sb, perm_e),
                (onesi_sb, onesi_e), (gam_sb, gam_e), (bet_sb, bet_e),
                (bo_sb, bo_e), (wq_sb, wq_e), (cq_sb, cq_e), (sq_sb, sq_e),
            ):
                nc.gpsimd.dma_start(t[:], e[:])

            x_sb = [
                [XP.tile([P, 512], F32, name=f"x{c}_{tq}") for tq in range(NQ)]
                for c in range(4)
            ]
            for tq in range(NQ):
                for c in range(4):
                    nc.sync.dma_start(x_sb[c][tq][:], x_e[c][:, ts(tq, 512)])

            # local-half rotary-k tables first, remote half + wo later
            nc.scalar.dma_start(ck_sb[:, 0:NLOC], ck_e[:, 0:NLOC])
            nc.scalar.dma_start(sk_sb[:, 0:NLOC], sk_e[:, 0:NLOC])
            nc.scalar.dma_start(ck_sb[:, NLOC:N], ck_e[:, NLOC:N])
            nc.scalar.dma_start(sk_sb[:, NLOC:N], sk_e[:, NLOC:N])
            nc.scalar.dma_start(wo_sb[:], wo_e[:])

            xnb = [PS.tile([P, N], BF16, name=f"xnb{c}") for c in range(4)]
            k_bf = PS.tile([P, N], BF16, name="kbf")
            v_loc = PS.tile([P, 2 * NCH, DH + 1], BF16, name="vloc")
            nc.gpsimd.memset(v_loc[:, :, DH : DH + 1], 1.0)
            qr_sb = [PS.tile([P, NLOC], BF16, name=f"qr{i}") for i in range(HP)]
            ohat = [PS.tile([P, NLOC], BF16, name=f"oh{i}") for i in range(HP)]

            # ---------------- per-quarter LN + K/V projection -------------
            xbf_all = [
                [XP.tile([P, 512], BF16, name=f"xb{c}_{tq}") for tq in range(NQ)]
                for c in range(4)
            ]
            bc_all = {}

            def emit_chain(pair):
                # stats + rstd/mrs chain for quarters (2*pair, 2*pair+1),
                # batched on [2, 512] rows. Uses ACT Copy/Sqrt -- pre-stream
                # only, so the exp table set is never evicted mid-stream.
                tqs = (2 * pair, 2 * pair + 1)
                xsq = {}
                for q01, tq in enumerate(tqs):
                    for c in range(4):
                        xbf = xbf_all[c][tq]
                        xq = TMP4.tile(
                            [P, 512], BF16, tag="xsq", name=f"xsq{c}_{tq}"
                        )
                        nc.vector.tensor_copy(xbf[:], x_sb[c][tq][:])
                        nc.scalar.activation(xq[:], x_sb[c][tq][:], AF.Square)
                        xsq[(c, q01)] = xq
                st = PSC.tile([P, 3, 512], F32, tag="sc", name="stats")
                for q01 in range(2):
                    for c in range(4):
                        nc.tensor.matmul(
                            st[0:65, 0, :], onesi_sb[:, q01, :],
                            xbf_all[c][tqs[q01]][:],
                            start=(q01 == 0 and c == 0),
                            stop=(q01 == 1 and c == 3),
                        )
                for q01 in range(2):
                    for c in range(4):
                        nc.tensor.matmul(
                            st[0:65, 1, :], onesi_sb[:, q01, :], xsq[(c, q01)][:],
                            start=(q01 == 0 and c == 0),
                            stop=(q01 == 1 and c == 3),
                        )
                mu2 = TMP4.tile([65, 512], F32, tag="ln")
                musq = TMP4.tile([65, 512], F32, tag="ln")
                var = TMP4.tile([65, 512], F32, tag="ln")
                rvar = TMP4.tile([65, 512], F32, tag="ln")
                rstd2 = TMP4.tile([65, 512], BF16, tag="lnb")
                mrs2 = TMP4.tile([65, 512], BF16, tag="lnb")
                nc.scalar.copy(mu2[:], st[0:65, 0, :])
                nc.vector.tensor_mul(musq[:], mu2[:], mu2[:])
                nc.vector.tensor_tensor(var[:], st[0:65, 1, :], musq[:], ALU.subtract)
                nc.vector.reciprocal_approx_fast(rvar[:], var[:])
                nc.scalar.activation(rstd2[:], rvar[:], AF.Sqrt)
                nc.vector.tensor_mul(mrs2[:], mu2[:], rstd2[:])
                for q01, tq in enumerate(tqs):
                    if q01 == 0:
                        rsrc, msrc = rstd2[0:1, :], mrs2[0:1, :]
                    else:
                        # partition 64 -> 0 staging (proven DVE shift copy)
                        rsrc = TMP4.tile([1, 512], BF16, tag="lnr")
                        msrc = TMP4.tile([1, 512], BF16, tag="lnr")
                        nc.vector.tensor_copy(rsrc[:], rstd2[64:65, :])
                        nc.vector.tensor_copy(msrc[:], mrs2[64:65, :])
                        rsrc, msrc = rsrc[:], msrc[:]
                    rstd_bc = XP.tile([P, 512], BF16, name=f"rstd{tq}")
                    mrs_bc = XP.tile([P, 512], BF16, name=f"mrs{tq}")
                    nc.gpsimd.partition_broadcast(rstd_bc[:], rsrc)
                    nc.gpsimd.partition_broadcast(mrs_bc[:], msrc)
                    bc_all[tq] = (rstd_bc, mrs_bc)

            def emit_xn(tq):
                # xn for a quarter (DVE only -- never stalls the PE queue)
                xbf = [xbf_all[c][tq] for c in range(4)]
                rstd_bc, mrs_bc = bc_all[tq]
                for c in range(4):
                    t1 = TMP.tile([P, 512], BF16, tag="xt")
                    nc.vector.tensor_mul(t1[:], xbf[c][:], rstd_bc[:])
                    if TRIVIAL_GB:
                        nc.vector.tensor_tensor(
                            xnb[c][:, ts(tq, 512)], t1[:], mrs_bc[:], ALU.subtract
                        )
                    else:
                        t2 = TMP.tile([P, 512], BF16, tag="xt")
                        nc.vector.tensor_tensor(t2[:], t1[:], mrs_bc[:], ALU.subtract)
                        nc.vector.tensor_scalar(
                            xnb[c][:, ts(tq, 512)], t2[:],
                            gam_sb[:, c : c + 1], bet_sb[:, c : c + 1],
                            ALU.mult, ALU.add,
                        )

            def emit_kv(tq):
                # k/v projection for a quarter; vproj matmuls run between the
                # k0 cast and the perm matmul so the PE never waits on DVE
                kp = PSC.tile([P, 3, 512], F32, tag="sc", name="kproj")
                for c in range(4):
                    nc.tensor.matmul(
                        kp[:, 0, :], wk_sb[:, c, :], xnb[c][:, ts(tq, 512)],
                        start=(c == 0), stop=(c == 3),
                    )
                k0 = TMP.tile([P, 512], BF16, tag="kt")
                nc.vector.tensor_copy(k0[:], kp[:, 0, :])
                vt = PSC.tile([P, 3, 512], F32, tag="sc", name="vproj")
                for vj in range(4):
                    for c in range(4):
                        nc.tensor.matmul(
                            vt[:, 0, ds(128 * vj, 128)],
                            xnb[c][:, ts(4 * tq + vj, 128)], wv_sb[:, c, :],
                            start=(c == 0), stop=(c == 3),
                        )
                nc.tensor.matmul(kp[:, 1, :], perm_sb[:], k0[:], start=True, stop=True)
                for vj in range(4):
                    for g in range(2):
                        nc.vector.tensor_copy(
                            v_loc[:, 2 * (4 * tq + vj) + g, 0:DH],
                            vt[:, 0, ds(128 * vj + 64 * g, DH)],
                        )
                ks = TMP.tile([P, 512], BF16, tag="kt")
                nc.vector.tensor_copy(ks[:], kp[:, 1, :])
                t1 = TMP.tile([P, 512], BF16, tag="kr")
                t2 = TMP.tile([P, 512], BF16, tag="kr")
                nc.vector.tensor_mul(t1[:], ck_sb[:, ts(tq, 512)], k0[:])
                nc.vector.tensor_mul(t2[:], sk_sb[:, ts(tq, 512)], ks[:])
                nc.vector.tensor_add(k_bf[:, ts(tq, 512)], t1[:], t2[:])

            def emit_qproj(hp):
                tps = []
                for tql in range(2):
                    t = PSC.tile([P, 3, 512], F32, tag="sc", name="qps")
                    for c in range(4):
                        nc.tensor.matmul(
                            t[:, 0, :], wq_sb[:, c, ts(hp, 128)],
                            xnb[c][:, ts(tql, 512)],
                            start=(c == 0), stop=(c == 3),
                        )
                    qc = TMP4.tile([P, 512], BF16, tag="qcs")
                    nc.vector.tensor_copy(qc[:], t[:, 0, :])
                    tps.append((t, qc))
                for tql, (t, qc) in enumerate(tps):
                    nc.tensor.matmul(
                        t[:, 1, :], perm_sb[:], qc[:], start=True, stop=True
                    )
                for tql, (t, qc) in enumerate(tps):
                    qs = TMP4.tile([P, 512], BF16, tag="qcs")
                    nc.vector.tensor_copy(qs[:], t[:, 1, :])
                    t1 = TMP.tile([P, 512], BF16, tag="qr")
                    t2 = TMP.tile([P, 512], BF16, tag="qr")
                    nc.vector.tensor_mul(t1[:], cq_sb[:, ts(tql, 512)], qc[:])
                    nc.vector.tensor_mul(t2[:], sq_sb[:, ts(tql, 512)], qs[:])
                    nc.vector.tensor_add(qr_sb[hp][:, ts(tql, 512)], t1[:], t2[:])

            def emit_outproj(mc, tql):
                t = PSC.tile([P, 3, 512], F32, tag="sc", name="yps")
                for kc in range(4):
                    nc.tensor.matmul(
                        t[:, 0, :], wo_sb[:, kc, ts(mc, 128)],
                        ohat[kc][:, ts(tql, 512)],
                        start=(kc == 0), stop=(kc == 3),
                    )
                yt = TMP.tile([P, 512], F32, tag="yout")
                nc.vector.tensor_add(yt[:], t[:, 0, :], xnb[mc][:, ts(tql, 512)])
                if TRIVIAL_BO:
                    yo = yt
                else:
                    yo = TMP.tile([P, 512], F32, tag="yout")
                    nc.vector.tensor_scalar_add(yo[:], yt[:], bo_sb[:, mc : mc + 1])
                nc.sync.dma_start(out_e[mc, :, ts(tql, 512)], yo[:])

            spills = {}

            def emit_spill(hp, tq, oA, oB):
                cpA = PS.tile([DH + 1, 512], F32, name=f"spA{hp}{tq}")
                cpB = PS.tile([DH + 1, 512], F32, name=f"spB{hp}{tq}")
                nc.vector.tensor_copy(cpA[:], oA[:])
                nc.vector.tensor_copy(cpB[:], oB[:])
                spills[(hp, tq)] = (cpA, cpB)

            def emit_epilogue(hp, tq, oA, oB, restore):
                # copy/merge out of PSUM first -- frees the AV bank so the
                # next block's AV matmuls are not stalled behind the divide
                sA = TMP.tile([DH + 1, 512], F32, tag="sum")
                sB = TMP.tile([DH + 1, 512], F32, tag="sum")
                if restore:
                    cpA, cpB = spills[(hp, tq)]
                    nc.vector.tensor_add(sA[:], oA[:], cpA[:])
                    nc.vector.tensor_add(sB[:], oB[:], cpB[:])
                else:
                    nc.vector.tensor_copy(sA[:], oA[:])
                    nc.vector.tensor_copy(sB[:], oB[:])
                den2 = TMP.tile([1, 1024], F32, tag="den")
                rec2 = TMP.tile([1, 1024], F32, tag="den")
                nc.vector.tensor_copy(den2[0:1, 0:512], sA[DH : DH + 1, :])
                nc.vector.tensor_copy(den2[0:1, 512:1024], sB[DH : DH + 1, :])
                nc.vector.reciprocal_approx_fast(rec2[:], den2[:])
                pbA = TMP.tile([64, 512], F32, tag="pb")
                pbB = TMP.tile([64, 512], F32, tag="pb")
                nc.gpsimd.partition_broadcast(pbA[:], rec2[0:1, 0:512])
                nc.gpsimd.partition_broadcast(pbB[:], rec2[0:1, 512:1024])
                nc.vector.tensor_mul(ohat[hp][0:64, ts(tq, 512)], sA[0:DH, :], pbA[:])
                nc.vector.tensor_mul(ohat[hp][64:128, ts(tq, 512)], sB[0:DH, :], pbB[:])

            def run_stream(plan, hooks):
                """plan: (hp, tq, chunks, mode) with mode spill/epi/epi_restore.
                Flat pipeline: scores(i) | AV(i-1) | exp(i). hooks keyed by
                global group index."""
                stream = []
                for hp, tq, chunks, mode in plan:
                    slots = [(par, c) for c in chunks for par in range(2)]
                    grps = [slots[i : i + 3] for i in range(0, len(slots), 3)]
                    for gi, g in enumerate(grps):
                        stream.append(
                            (hp, tq, g, chunks[0], chunks[-1],
                             gi == 0, gi == len(grps) - 1, mode)
                        )
                prev = None
                avt = {}
                for idx, it in enumerate(stream + [None]):
                    for hk in hooks.get(idx, []):
                        hk()
                    if it is not None:
                        hp, tq, grp, _, _, _, _, _ = it
                        sc = PSC.tile([P, 3, 512], F32, tag="sc")
                        for pos, (par, c) in enumerate(grp):
                            nc.tensor.matmul(
                                sc[:, pos, :],
                                k_bf[64 * par : 64 * (par + 1), ts(c, 128)],
                                qr_sb[hp][64 * par : 64 * (par + 1), ts(tq, 512)],
                                start=True, stop=True,
                                tile_position=(64 * par, 0),
                            )
                    if prev is not None:
                        ep, (php, ptq, pgrp, c0, c1, pfirst, plast, pmode) = prev
                        if pfirst:
                            av_a = PAV.tile([DH + 1, 512], F32, tag="avA", name="av_a")
                            av_b = PAV.tile([DH + 1, 512], F32, tag="avB", name="av_b")
                            avt[(php, ptq)] = (av_a, av_b)
                        oA, oB = avt[(php, ptq)]
                        for pos, (par, c) in enumerate(pgrp):
                            nc.tensor.matmul(
                                oA[:] if par == 0 else oB[:],
                                v_loc[:, 2 * c + par, :],
                                ep[:, ts(pos, 512)],
                                start=(c == c0), stop=(c == c1),
                            )
                        if plast:
                            oA, oB = avt.pop((php, ptq))
                            if pmode == "spill":
                                emit_spill(php, ptq, oA, oB)
                            else:
                                emit_epilogue(
                                    php, ptq, oA, oB, pmode == "epi_restore"
                                )
                    if it is not None:
                        e = EXPP.tile([P, 1536], BF16, tag="e")
                        nc.scalar.activation(
                            e[:, 0 : 512 * len(it[2])],
                            sc[:, 0 : len(it[2]), :].rearrange("p a b -> p (a b)"),
                            AF.Exp, scale=0.125,
                        )
                        prev = (e, it)

            # all LN chains + local-quarter K/V + first q head pair happen
            # before the stream starts
            emit_chain(0)
            emit_xn(0)
            emit_xn(1)
            emit_kv(0)
            emit_kv(1)
            emit_chain(1)
            emit_qproj(0)

            LOC = list(range(8))
            REM = list(range(8, 16))
            ALL = list(range(16))
            # block (0,1) runs its local chunks first (spilled) so the
            # stream starts before remote-quarter K/V exist; its remote
            # half finishes at the end.
            plan = [
                (0, 1, LOC, "spill"),
                (0, 0, ALL, "epi"), (1, 0, ALL, "epi"),
                (2, 0, ALL, "epi"), (3, 0, ALL, "epi"),
                (1, 1, ALL, "epi"), (2, 1, ALL, "epi"), (3, 1, ALL, "epi"),
                (0, 1, REM, "epi_restore"),
            ]
            # group counts: 6 for the 16-slot blocks, 11 for 32-slot blocks
            # starts: [0, 6, 17, 28, 39, 50, 61, 72, 83]
            hooks = {
                1: [lambda: emit_xn(2)],
                3: [lambda: emit_xn(3)],
                4: [lambda: emit_kv(2)],
                7: [lambda: emit_kv(3)],
                9: [lambda: emit_qproj(1)],
                19: [lambda: emit_qproj(2)],
                30: [lambda: emit_qproj(3)],
            }
            for j, mc in enumerate(range(4)):
                hooks.setdefault(52 + 6 * j, []).append(
                    (lambda m: lambda: emit_outproj(m, 0))(mc)
                )
            run_stream(plan, hooks)
            for mc in range(4):
                emit_outproj(mc, 1)

    nc.compile()
    return nc


def _host_inputs(x, gamma, beta, Wq, Wkv, Wout, bout):
    """Build the 8 per-core input maps."""
    x = np.asarray(x, np.float32)
    gamma = np.asarray(gamma, np.float32)
    beta = np.asarray(beta, np.float32)
    Wq = np.asarray(Wq, np.float32)
    Wkv = np.asarray(Wkv, np.float32)
    Wout = np.asarray(Wout, np.float32)
    bout = np.asarray(bout, np.float32)

    def lhsT(W):
        # [D, M] -> [128, 4, M] chunk layout
        return np.ascontiguousarray(
            W.reshape(4, P, W.shape[1]).transpose(1, 0, 2).astype(bfdt)
        )

    Wk = Wkv[:, : KVH * DH]
    Wv = Wkv[:, KVH * DH :]
    wq = lhsT(Wq)
    wk = lhsT(Wk)
    wv = lhsT(Wv)
    wo = lhsT(Wout)
    gam = np.ascontiguousarray(gamma.reshape(4, P).T)
    bet = np.ascontiguousarray(beta.reshape(4, P).T)
    bo = np.ascontiguousarray(bout.reshape(4, P).T)
    onesi = np.zeros((P, 2, 65), np.float32)
    onesi[:, 0, 0] = 1.0 / 512.0
    onesi[:, 1, 64] = 1.0 / 512.0
    onesi = onesi.astype(bfdt)
    # permutation matrix: row p has a 1 at column p^32 (swap 32-halves
    # within each 64-dim head)
    perm = np.zeros((P, P), np.float32)
    perm[np.arange(P), np.arange(P) ^ 32] = 1.0
    perm = perm.astype(bfdt)

    # rotary tables (position-dependent); j indexes the 128 stacked dims
    # (2 heads x 64)
    j = np.arange(DH)
    inv_freq = 1.0 / (10000.0 ** ((2.0 * (j % 32)) / DH))
    base = ((2.0 * (j % 32)) + 0.4 * DH) / (1.4 * DH)
    sign = np.where(j < 32, -1.0, 1.0)

    def tables(pos, is_q):
        freqs = pos[None, :] * inv_freq[:, None]  # [64, n]
        cos, sin = np.cos(freqs), np.sin(freqs)
        power = (pos - N // 2) / CONTEXT_LEN
        xsc = base[:, None] ** power[None, :]
        if is_q:
            c = np.tile(cos * xsc, (2, 1))
            s = np.tile(sign[:, None] * sin * xsc, (2, 1))
        else:
            c = np.tile(cos / xsc, (2, 1))
            s = np.tile(sign[:, None] * sin / xsc, (2, 1))
        return c.astype(bfdt), s.astype(bfdt)

    in_maps = []
    for core in range(NCORES):
        b, half = core // 2, core % 2
        # token order: local half first
        order = (np.arange(N) + half * NLOC) % N
        pos = order.astype(np.float64)
        xc = np.ascontiguousarray(x[b].reshape(4, P, N)[:, :, order])
        cq, sq = tables(pos[:NLOC], True)
        ck, sk = tables(pos, False)
        in_maps.append(
            {
                "x": xc, "wq": wq, "wk": wk, "wv": wv, "wo": wo,
                "perm": perm, "onesi": onesi,
                "cq": cq, "sq": sq, "ck": ck, "sk": sk,
                "gam": gam, "bet": bet, "bo": bo,
            }
        )
    return in_maps


def kernel(x, gamma, beta, Wq, Wkv, Wout, bout):
    trace = os.environ.get("KERNEL_TRACE", "0") == "1"
    if trace:
        _enable_trace_hook()
    trivial_gb = bool(
        np.all(np.asarray(gamma) == 1.0) and np.all(np.asarray(beta) == 0.0)
    )
    trivial_bo = bool(np.all(np.asarray(bout) == 0.0))
    if "nc" not in _CACHE:
        _CACHE["nc"] = _build(trivial_gb, trivial_bo)
        _CACHE["trivial_gb"] = (trivial_gb, trivial_bo)
    assert _CACHE["trivial_gb"] == (trivial_gb, trivial_bo)
    nc = _CACHE["nc"]
    in_maps = _host_inputs(x, gamma, beta, Wq, Wkv, Wout, bout)
    res = run_bass_kernel_spmd(nc, in_maps, list(range(NCORES)), trace=trace)
    if trace and res.exec_time_ns is not None:
        print(f"HW exec time: {res.exec_time_ns} ns")
        _CACHE["exec_time_ns"] = res.exec_time_ns

    y = np.empty((B, D, N), np.float32)
    for core in range(NCORES):
        b, half = core // 2, core % 2
        y[b, :, half * NLOC : (half + 1) * NLOC] = res.results[core]["out"].reshape(
            D, NLOC
        )
    return y
